# revision 39
# baseline (speedup 1.0000x reference)
"""Trainium2 Bass kernel for a top-2 MoE layer (B=2, T=2048, D=1024, F=4096, E=8).

Strategy (expert-parallel, per sharding hint):
  Launch 1 (router, data-parallel over tokens): each of 8 cores computes
    logits = x_slice @ Wr in fp32 on the PE (warmup matmuls ramp the PE
    clock while the x DMA lands across all 3 DMA queues), then top-2 +
    renormalized softmax combine weights batched on DVE/ACT.
  Host dispatch (data movement only): tokens are gathered per expert
    (all-to-all performed by the host), padded to a static capacity.
  Launch 2 (expert FFN, expert-parallel): core e holds expert e's W1/W2
    resident in SBUF as fp16 (half the HBM traffic of fp32), single pass
    over F per token chunk: y = c * (gelu(x@W1+b1) @ W2 [+ b2]).
  Launch 3 (combine): out[t] = yA[t] + yB[t] in fp16 — the two selected
    experts' scaled outputs per token added on DVE; host widens to fp32.

All arithmetic is on-device; the host only reshapes/gathers/concats
(and losslessly widens the final fp16 sums to fp32).
"""

import numpy as np

import concourse.bacc as bacc
import concourse.mybir as mybir
import concourse.tile as tile
from concourse import bass_utils

F32 = mybir.dt.float32
F32R = mybir.dt.float32r
F16 = mybir.dt.float16
AX = mybir.AxisListType
ALU = mybir.AluOpType
ACT_F = mybir.ActivationFunctionType

B, T, D, F, E = 2, 2048, 1024, 4096, 8
NTOK = B * T              # 4096
NCORES = 8
TOK_PER_CORE = NTOK // NCORES  # 512
DO = D // 128             # 8 d-blocks
FT = F // 128             # 32 f-tiles

_cache = {}


def _run(nc, in_maps, trace=False, **kw):
    return bass_utils.run_bass_kernel_spmd(
        nc, in_maps, core_ids=list(range(NCORES)), trace=trace, **kw
    )


# ----------------------------------------------------------------- router ---
def _build_router_inner(batched):
    """Per core: xT_sl [128, DO*512] fp32, Wr [128, DO*E], id8 [E,E]
    -> comb [128, TT*E] fp32 (comb[p, tt*E+e] for token tt*128+p)."""
    TT = TOK_PER_CORE // 128  # 4 token tiles
    nc = bacc.Bacc("TRN2", target_bir_lowering=False, debug=False)
    xT_d = nc.dram_tensor("xT_sl", [128, DO * TOK_PER_CORE], F32,
                          kind="ExternalInput").ap()
    wr_d = nc.dram_tensor("Wr", [128, DO * E], F32, kind="ExternalInput").ap()
    id_d = nc.dram_tensor("id8", [E, E], F32, kind="ExternalInput").ap()
    out_d = nc.dram_tensor("comb", [128, TT * E], F32, kind="ExternalOutput").ap()

    with tile.TileContext(nc) as tc:
        with (
            tc.tile_pool(name="pool", bufs=1) as pool,
            tc.tile_pool(name="work", bufs=2) as work,
            tc.tile_pool(name="psum", bufs=2, space="PSUM") as psum,
            tc.tile_pool(name="psw", bufs=1, space="PSUM") as psw,
        ):
            xT_sb = pool.tile([128, DO, TOK_PER_CORE], F32)
            wr_sb = pool.tile([128, DO, E], F32)
            id_sb = pool.tile([E, E], F32)
            warm_sb = pool.tile([128, 512], F16)
            comb_sb = pool.tile([128, TT, E], F32)

            # x load split across both HW DMA queues (5/3: the scalar queue
            # starts ~2.5us later behind the ACT table load); small on SW
            xv = xT_d.rearrange("p (o t) -> p o t", o=DO)
            nc.gpsimd.memset(warm_sb[:], 0.0)
            nc.sync.dma_start(xT_sb[:, 0:5, :], xv[:, 0:5, :])
            nc.scalar.dma_start(xT_sb[:, 5:8, :], xv[:, 5:8, :])
            nc.gpsimd.dma_start(wr_sb[:], wr_d.rearrange("p (o e) -> p o e", o=DO))
            nc.gpsimd.dma_start(id_sb[:], id_d[:])

            # PE clock ramp while DMAs land — one accumulation chain so the
            # matmuls run back-to-back (separate start/stop warms serialize
            # on the PSUM WAR and never establish continuous-busy)
            warm_ps = psw.tile([128, 512], F32)
            NWARM = 20
            for i in range(NWARM):
                nc.tensor.matmul(warm_ps[:], warm_sb[:, :128], warm_sb[:],
                                 start=(i == 0), stop=(i == NWARM - 1))

            # logits.T [E, tok] with Wr stationary, then PE-transpose each
            # 128-token tile back to [tok, E]
            lpT = psum.tile([E, TOK_PER_CORE], F32, tag="lpT")
            for do in range(DO):
                nc.tensor.matmul(
                    lpT[:], wr_sb[:, do, :], xT_sb[:, do, :],
                    start=(do == 0), stop=(do == DO - 1),
                )
            lsbT = pool.tile([E, TOK_PER_CORE], F32)
            nc.vector.tensor_copy(lsbT[:], lpT[:])

            l4 = pool.tile([128, TT, E], F32)
            for tt in range(TT):
                lp = psum.tile([128, E], F32, tag="lp")
                nc.tensor.transpose(
                    lp[:], lsbT[:, tt * 128:(tt + 1) * 128], id_sb[:]
                )
                nc.vector.tensor_copy(l4[:, tt, :], lp[:])

            if batched:
                # top-2 + renormalized softmax over all 4 token tiles at once
                shp = [128, TT, E]
                mx1 = work.tile([128, TT, 1], F32, tag="mx1")
                nc.vector.reduce_max(mx1[:], l4[:], axis=AX.X)
                mx1b = mx1[:].broadcast_to(shp)
                eq1 = work.tile(shp, F32, tag="eq1")
                nc.vector.tensor_tensor(eq1[:], l4[:], mx1b, op=ALU.is_equal)
                lm = work.tile(shp, F32, tag="lm")
                nc.vector.scalar_tensor_tensor(
                    lm[:], eq1[:], -1e30, l4[:], op0=ALU.mult, op1=ALU.add
                )
                mx2 = work.tile([128, TT, 1], F32, tag="mx2")
                nc.vector.reduce_max(mx2[:], lm[:], axis=AX.X)
                pd = work.tile(shp, F32, tag="pd")
                nc.vector.tensor_tensor(pd[:], l4[:], mx1b, op=ALU.subtract)
                p = work.tile(shp, F32, tag="p")
                nc.scalar.activation(p[:], pd[:], ACT_F.Exp)
                e2d = work.tile([128, TT, 1], F32, tag="e2d")
                nc.vector.tensor_tensor(e2d[:], mx2[:], mx1[:], op=ALU.subtract)
                e2 = work.tile([128, TT, 1], F32, tag="e2")
                nc.scalar.activation(e2[:], e2d[:], ACT_F.Exp)
                den = work.tile([128, TT, 1], F32, tag="den")
                nc.vector.tensor_scalar_add(den[:], e2[:], 1.0)
                rec = work.tile([128, TT, 1], F32, tag="rec")
                nc.vector.reciprocal(rec[:], den[:])
                ge = work.tile(shp, F32, tag="ge")
                nc.vector.tensor_tensor(ge[:], l4[:], mx2[:].broadcast_to(shp),
                                        op=ALU.is_ge)
                w = work.tile(shp, F32, tag="w")
                nc.vector.tensor_tensor(w[:], p[:], rec[:].broadcast_to(shp),
                                        op=ALU.mult)
                nc.vector.tensor_mul(comb_sb[:], w[:], ge[:])
            else:
                for tt in range(TT):
                    l = l4[:, tt, :]
                    mx1 = work.tile([128, 1], F32, tag="mx1")
                    nc.vector.reduce_max(mx1[:], l, axis=AX.X)
                    nmx1 = work.tile([128, 1], F32, tag="nmx1")
                    nc.vector.tensor_scalar_mul(nmx1[:], mx1[:], -1.0)
                    eq = work.tile([128, E], F32, tag="eq")
                    nc.vector.tensor_scalar(eq[:], l, mx1[:], None, op0=ALU.is_equal)
                    lm = work.tile([128, E], F32, tag="lm")
                    nc.vector.scalar_tensor_tensor(
                        lm[:], eq[:], -1e30, l, op0=ALU.mult, op1=ALU.add
                    )
                    mx2 = work.tile([128, 1], F32, tag="mx2")
                    nc.vector.reduce_max(mx2[:], lm[:], axis=AX.X)
                    p = work.tile([128, E], F32, tag="p")
                    nc.scalar.activation(p[:], l, ACT_F.Exp, bias=nmx1[:])
                    e2 = work.tile([128, 1], F32, tag="e2")
                    nc.scalar.activation(e2[:], mx2[:], ACT_F.Exp, bias=nmx1[:])
                    den = work.tile([128, 1], F32, tag="den")
                    nc.vector.tensor_scalar_add(den[:], e2[:], 1.0)
                    rec = work.tile([128, 1], F32, tag="rec")
                    nc.vector.reciprocal(rec[:], den[:])
                    ge = work.tile([128, E], F32, tag="ge")
                    nc.vector.tensor_scalar(ge[:], l, mx2[:], None, op0=ALU.is_ge)
                    w = work.tile([128, E], F32, tag="w")
                    nc.vector.tensor_scalar_mul(w[:], p[:], rec[:])
                    nc.vector.tensor_mul(comb_sb[:, tt, :], w[:], ge[:])

            nc.sync.dma_start(out_d.rearrange("p (t e) -> p t e", t=TT), comb_sb[:])
    nc.compile()
    return nc


def build_router():
    if "router" in _cache:
        return _cache["router"]
    try:
        nc = _build_router_inner(batched=True)
    except Exception:
        nc = _build_router_inner(batched=False)
    _cache["router"] = nc
    return nc


# -------------------------------------------------------------------- ffn ---
def build_ffn(cap, with_b1, with_b2):
    """Per core (expert e), all matmul operands fp16, accumulate fp32:
      xg [128, DO*cap], W1 [128, FT, DO, 128], W2 [128, FT, D] (+ b1/b2)
      -> y [128, TT, D] fp16 with y = cvec * (gelu(xg@W1 + b1) @ W2 + b2).
    Weights fully resident in SBUF; single pass over F per token chunk."""
    key = ("ffn", cap, with_b1, with_b2)
    if key in _cache:
        return _cache[key]
    assert cap % 32 == 0
    chunks = _chunk_split(cap)
    CH = max(cs for _, cs in chunks)
    TT = -(-cap // 128)

    nc = bacc.Bacc("TRN2", target_bir_lowering=False, debug=False)
    xg_d = nc.dram_tensor("xg", [128, DO * cap], F16, kind="ExternalInput").ap()
    w1_d = nc.dram_tensor("W1e", [128, FT, DO, 128], F16, kind="ExternalInput").ap()
    w2_d = nc.dram_tensor("W2e", [128, FT, D], F16, kind="ExternalInput").ap()
    b1_d = nc.dram_tensor("b1e", [128, FT], F32, kind="ExternalInput").ap()
    b2_d = nc.dram_tensor("b2e", [1, D], F16, kind="ExternalInput").ap()
    ones_d = nc.dram_tensor("ones", [1, 128], F16, kind="ExternalInput").ap()
    cv_d = nc.dram_tensor("cvec", [128, TT], F32, kind="ExternalInput").ap()
    y_d = nc.dram_tensor("y", [128, TT, D], F16, kind="ExternalOutput").ap()

    with tile.TileContext(nc) as tc:
        with (
            tc.tile_pool(name="res", bufs=1) as res,
            tc.tile_pool(name="outp", bufs=2) as outp,
            tc.tile_pool(name="ps1", bufs=4, space="PSUM") as ps1,
            tc.tile_pool(name="ps2", bufs=1, space="PSUM") as ps2,
        ):
            xg_sb = res.tile([128, DO * cap], F16)
            w1_sb = res.tile([128, FT, DO, 128], F16)
            w2_sb = res.tile([128, FT, D], F16)
            hT_sb = res.tile([128, FT, CH], F16)
            b1_sb = res.tile([128, FT], F32)
            b2_sb = res.tile([1, D], F16)
            ones_sb = res.tile([1, 128], F16)
            cv_sb = res.tile([128, TT], F32)
            warm_sb = res.tile([128, 512], F16)

            # --- input DMA schedule. Caution: dma_start BLOCKS the issuing
            # engine while its queue is full, and the scalar engine must be
            # free to run gelu from ~15us on — so scalar gets only 2 up-front
            # issues; most of W2 is issued from inside the chunk-0 loop,
            # paced between gelus. 4-ft W1 pieces give 8 KiB/partition
            # packets (~250 GB/s per HW queue), streaming just ahead of
            # chunk-0 stage 1.
            nc.gpsimd.memset(warm_sb[:], 0.0)
            cs0 = chunks[0][1]
            half = 4 * cs0
            nc.sync.dma_start(xg_sb[:, :half], xg_d[:, :half])
            nc.scalar.dma_start(xg_sb[:, half:2 * half], xg_d[:, half:2 * half])
            nc.sync.dma_start(w1_sb[:, 0:2], w1_d[:, 0:2])
            nc.scalar.dma_start(w1_sb[:, 2:4], w1_d[:, 2:4])
            nc.scalar.dma_start(w1_sb[:, 4:6], w1_d[:, 4:6])
            for k in [6, 8, 12, 16, 20, 24, 28]:
                nc.sync.dma_start(w1_sb[:, k:k + 2], w1_d[:, k:k + 2])
            nc.gpsimd.dma_start(cv_sb[:], cv_d[:])
            if with_b1:
                nc.gpsimd.dma_start(b1_sb[:], b1_d[:])
            if with_b2:
                nc.gpsimd.dma_start(b2_sb[:], b2_d[:])
                nc.gpsimd.dma_start(ones_sb[:], ones_d[:])
            for k in [10, 14, 18, 22, 26, 30]:
                nc.gpsimd.dma_start(w1_sb[:, k:k + 2], w1_d[:, k:k + 2])
            # tails: remaining x chunks and the last W2 pieces
            nc.sync.dma_start(w2_sb[:, 20:26], w2_d[:, 20:26])
            for c0, cs in chunks[1:]:
                nc.gpsimd.dma_start(
                    xg_sb[:, DO * c0:DO * (c0 + cs)], xg_d[:, DO * c0:DO * (c0 + cs)]
                )
            nc.gpsimd.dma_start(w2_sb[:, 26:FT], w2_d[:, 26:FT])

            # PE clock ramp while the first W1/xg DMAs land — single
            # accumulation chain for back-to-back execution (see router)
            warm_ps = ps1.tile([128, 512], F32, tag="hp", name="warm_ps",
                               bufs=4)
            NWARM = 28
            for i in range(NWARM):
                nc.tensor.matmul(warm_ps[:], warm_sb[:, :128], warm_sb[:],
                                 start=(i == 0), stop=(i == NWARM - 1))

            for ci, (c0, cs) in enumerate(chunks):
                ntt = -(-cs // 128)
                # stage 1: hT[f, tok] = gelu(W1.T @ x + b1), all of F
                for ft in range(FT):
                    hp = ps1.tile([128, 512], F32, tag="hp")
                    for do in range(DO):
                        nc.tensor.matmul(
                            hp[:, :cs],
                            w1_sb[:, ft, do, :],
                            xg_sb[:, DO * c0 + do * cs:DO * c0 + (do + 1) * cs],
                            start=(do == 0),
                            stop=(do == DO - 1),
                        )
                    if with_b1:
                        nc.scalar.activation(hT_sb[:, ft, :cs], hp[:, :cs],
                                             ACT_F.Gelu, bias=b1_sb[:, ft:ft + 1])
                    else:
                        nc.scalar.activation(hT_sb[:, ft, :cs], hp[:, :cs],
                                             ACT_F.Gelu)
                    # paced W2 issues on the (otherwise idle-ish) scalar
                    # engine during chunk-0 stage 1 — never >2 in flight
                    if ci == 0 and ft in (4, 8, 12, 16, 20):
                        nc.scalar.dma_start(w2_sb[:, ft - 4:ft],
                                            w2_d[:, ft - 4:ft])
                # stage 2: y[tok, d] = cvec * (hT.T @ W2 (+ b2)), fo-major over
                # pairs of token tiles so W2[fo] is consumed at half the rate
                # (relaxes the W2 streaming deadline on chunk 0)
                tiles = list(range(ntt))
                groups = [tiles[i:i + 2] for i in range(0, ntt, 2)]
                for group in groups:
                    yps, ms = [], []
                    for gi, tt in enumerate(group):
                        yps.append(ps2.tile([128, D], F32, tag=f"yp{gi}",
                                            name=f"yp{gi}"))
                        ms.append(min(128, cs - tt * 128))
                    for fo in range(FT):
                        for gi, tt in enumerate(group):
                            for n in range(2):
                                nc.tensor.matmul(
                                    yps[gi][:ms[gi], n * 512:(n + 1) * 512],
                                    hT_sb[:, fo, tt * 128:tt * 128 + ms[gi]],
                                    w2_sb[:, fo, n * 512:(n + 1) * 512],
                                    start=(fo == 0),
                                    stop=(fo == FT - 1 and not with_b2),
                                )
                    for gi, tt in enumerate(group):
                        m = ms[gi]
                        if with_b2:
                            for n in range(2):
                                nc.tensor.matmul(
                                    yps[gi][:m, n * 512:(n + 1) * 512],
                                    ones_sb[:, :m],
                                    b2_sb[:, n * 512:(n + 1) * 512],
                                    start=False,
                                    stop=True,
                                )
                        gt = c0 // 128 + tt
                        ot = outp.tile([128, D], F16, tag="ot")
                        nc.vector.tensor_scalar_mul(ot[:m, :], yps[gi][:m, :],
                                                    cv_sb[:m, gt:gt + 1])
                        nc.sync.dma_start(y_d[:m, gt, :], ot[:m, :])
    nc.compile()
    _cache[key] = nc
    return nc


# ---------------------------------------------------------------- combine ---
def build_combine():
    """Per core: packed a, b [128, (T/128)*D] fp16 -> o = a + b (fp16).

    Host packs A[t, d] -> Ah[p, tt*D + d] with t = tt*128 + p so every DMA is
    one contiguous segment per partition."""
    if "comb" in _cache:
        return _cache["comb"]
    W = (TOK_PER_CORE // 128) * D  # 4096
    PW = W // 2
    HP = PW // 2
    nc = bacc.Bacc("TRN2", target_bir_lowering=False, debug=False)
    a_d = nc.dram_tensor("a", [128, W], F16, kind="ExternalInput").ap()
    b_d = nc.dram_tensor("b", [128, W], F16, kind="ExternalInput").ap()
    o_d = nc.dram_tensor("o", [128, W], F16, kind="ExternalOutput").ap()
    with tile.TileContext(nc) as tc:
        with tc.tile_pool(name="pool", bufs=2) as pool:
            # 2 big read pieces per tensor (4 KiB/partition packets keep the
            # HW queues at full rate); adds per piece; outs on all 3 queues
            tiles = []
            for pc in range(2):
                sl = slice(pc * PW, (pc + 1) * PW)
                at = pool.tile([128, PW], F16, tag="a")
                bt = pool.tile([128, PW], F16, tag="b")
                nc.sync.dma_start(at[:], a_d[:, sl])
                nc.scalar.dma_start(bt[:], b_d[:, sl])
                nc.vector.tensor_add(at[:], at[:], bt[:])
                tiles.append(at)
            # outs on the HW queues only — the SWDGE path adds ~2-3us of
            # software descriptor latency right on the critical tail
            nc.sync.dma_start(o_d[:, 0:HP], tiles[0][:, 0:HP])
            nc.scalar.dma_start(o_d[:, HP:PW], tiles[0][:, HP:PW])
            nc.sync.dma_start(o_d[:, PW:PW + HP], tiles[1][:, 0:HP])
            nc.scalar.dma_start(o_d[:, PW + HP:W], tiles[1][:, HP:PW])
    nc.compile()
    _cache["comb"] = nc
    return nc


# ----------------------------------------------------------------- driver ---
def _chunk_split(cap):
    """Split cap (multiple of 32) into chunks: all 128-aligned starts, sizes
    multiples of 128 except the last (multiple of 32), each <=512."""
    full = cap // 128
    rem = cap % 128
    k = -(-cap // 512)
    counts = [full // k + (1 if i < full % k else 0) for i in range(k)]
    chunks, c0 = [], 0
    for i, n in enumerate(counts):
        cs = n * 128 + (rem if i == k - 1 else 0)
        chunks.append((c0, cs))
        c0 += cs
    return chunks


def _moe_forward(x2d, Wr, W1, b1, W2, b2, trace=False):
    """x2d: [NTOK, D] fp32. Returns (out [NTOK, D] fp32, exec_ns_total|None)."""
    # --- launch 1: router ---
    rnc = build_router()
    wrh = np.ascontiguousarray(Wr.reshape(DO, 128, E).transpose(1, 0, 2).reshape(128, -1))
    id8 = np.eye(E, dtype=np.float32)
    in_maps = [
        {"xT_sl": np.ascontiguousarray(
            x2d[c * TOK_PER_CORE:(c + 1) * TOK_PER_CORE]
            .reshape(TOK_PER_CORE, DO, 128).transpose(2, 1, 0).reshape(128, -1)),
         "Wr": wrh, "id8": id8}
        for c in range(NCORES)
    ]
    rres = _run(rnc, in_maps, trace=trace)
    comb = np.concatenate(
        [rres.results[c]["comb"].reshape(128, TOK_PER_CORE // 128, E)
         .transpose(1, 0, 2).reshape(TOK_PER_CORE, E) for c in range(NCORES)],
        axis=0)
    exec_ns = rres.exec_time_ns or 0
    per_launch = [rres.exec_time_ns]

    # --- host dispatch (data movement only) ---
    x16 = x2d.astype(np.float16)
    top2 = np.argpartition(-comb, 1, axis=1)[:, :2]  # [NTOK, 2]
    sel_lists, cvals = [], []
    for e in range(E):
        sel = np.nonzero((top2 == e).any(axis=1))[0]
        sel_lists.append(sel)
        cvals.append(comb[sel, e])
    counts = np.array([len(s) for s in sel_lists])
    MAXCAP = 2048  # SBUF limit for resident x + weights
    nbatch = max(1, -(-int(counts.max()) // MAXCAP))
    cap = int(max(256, -(-(-(-counts.max() // nbatch)) // 32) * 32))

    with_b1 = bool(np.any(b1))
    with_b2 = bool(np.any(b2))
    fnc = build_ffn(cap, with_b1, with_b2)
    chunks = _chunk_split(cap)
    TTC = -(-cap // 128)
    ones_in = np.ones((1, 128), np.float16)
    w_packed = [
        {"W1e": np.ascontiguousarray(
            W1[e].astype(np.float16).reshape(DO, 128, FT, 128)
            .transpose(1, 2, 0, 3)),
         "b1e": np.ascontiguousarray(b1[e].reshape(FT, 128).T),
         "W2e": np.ascontiguousarray(
            W2[e].astype(np.float16).reshape(FT, 128, D).transpose(1, 0, 2)),
         "b2e": np.ascontiguousarray(b2[e].astype(np.float16)).reshape(1, D)}
        for e in range(E)
    ]
    ys = [np.zeros((0, D), np.float16) for _ in range(E)]
    for bi in range(nbatch):
        in_maps = []
        for e in range(E):
            sel_b = sel_lists[e][bi * cap:(bi + 1) * cap]
            cv_b = cvals[e][bi * cap:(bi + 1) * cap]
            n_e = len(sel_b)
            xsel = np.zeros((cap, D), np.float16)
            xsel[:n_e] = x16[sel_b]
            xg = np.concatenate(
                [xsel[c0:c0 + cs].reshape(cs, DO, 128).transpose(2, 1, 0)
                 .reshape(128, -1) for (c0, cs) in chunks], axis=1)
            cv = np.zeros(TTC * 128, np.float32)
            cv[:n_e] = cv_b
            cv = np.ascontiguousarray(cv.reshape(TTC, 128).T)
            in_maps.append({"xg": np.ascontiguousarray(xg), "ones": ones_in,
                            "cvec": cv, **w_packed[e]})
        fres = _run(fnc, in_maps, trace=trace)
        ys = [np.concatenate(
            [ys[e],
             fres.results[e]["y"].reshape(128, TTC, D).transpose(1, 0, 2)
             .reshape(TTC * 128, D)[:cap]]) for e in range(E)]
        exec_ns += fres.exec_time_ns or 0
        per_launch.append(fres.exec_time_ns)

    # --- host: build per-token (A, B) contribution rows (gather only) ---
    slot = np.zeros((NTOK, E), np.int64)
    for e in range(E):
        slot[sel_lists[e], e] = np.arange(counts[e])
    e1, e2v = top2[:, 0], top2[:, 1]
    A = np.empty((NTOK, D), np.float16)
    Bm = np.empty((NTOK, D), np.float16)
    for e in range(E):
        m1 = e1 == e
        A[m1] = ys[e][slot[m1, e]]
        m2 = e2v == e
        Bm[m2] = ys[e][slot[m2, e]]

    # --- launch 3: combine ---
    cnc = build_combine()

    def pack(m, c):
        sl = m[c * TOK_PER_CORE:(c + 1) * TOK_PER_CORE]
        return np.ascontiguousarray(
            sl.reshape(TOK_PER_CORE // 128, 128, D).transpose(1, 0, 2)
            .reshape(128, -1))

    in_maps = [{"a": pack(A, c), "b": pack(Bm, c)} for c in range(NCORES)]
    cres = _run(cnc, in_maps, trace=trace)
    out = np.concatenate(
        [cres.results[c]["o"].reshape(128, TOK_PER_CORE // 128, D)
         .transpose(1, 0, 2).reshape(TOK_PER_CORE, D) for c in range(NCORES)],
        axis=0).astype(np.float32)
    exec_ns += cres.exec_time_ns or 0
    per_launch.append(cres.exec_time_ns)
    if trace:
        print(f"per-launch exec ns (router, ffn, combine): {per_launch}")
        _moe_forward.last = (rres, fres, cres)
    return out, (exec_ns if trace else None)


def kernel(x, Wr, W1, b1, W2, b2):
    x = np.asarray(x, np.float32)
    out, _ = _moe_forward(
        x.reshape(NTOK, D),
        np.asarray(Wr, np.float32),
        np.asarray(W1, np.float32),
        np.asarray(b1, np.float32),
        np.asarray(W2, np.float32),
        np.asarray(b2, np.float32),
        trace=False,
    )
    return out.reshape(B, T, D)


# revision 40
# speedup vs baseline: 1.0041x; 1.0041x over previous
"""Trainium2 Bass kernel for a top-2 MoE layer (B=2, T=2048, D=1024, F=4096, E=8).

Strategy (expert-parallel, per sharding hint):
  Launch 1 (router, data-parallel over tokens): each of 8 cores computes
    logits = x_slice @ Wr in fp32 on the PE (warmup matmuls ramp the PE
    clock while the x DMA lands across all 3 DMA queues), then top-2 +
    renormalized softmax combine weights batched on DVE/ACT.
  Host dispatch (data movement only): tokens are gathered per expert
    (all-to-all performed by the host), padded to a static capacity.
  Launch 2 (expert FFN, expert-parallel): core e holds expert e's W1/W2
    resident in SBUF as fp16 (half the HBM traffic of fp32), single pass
    over F per token chunk: y = c * (gelu(x@W1+b1) @ W2 [+ b2]).
  Launch 3 (combine): out[t] = yA[t] + yB[t] in fp16 — the two selected
    experts' scaled outputs per token added on DVE; host widens to fp32.

All arithmetic is on-device; the host only reshapes/gathers/concats
(and losslessly widens the final fp16 sums to fp32).
"""

import numpy as np

import concourse.bacc as bacc
import concourse.mybir as mybir
import concourse.tile as tile
from concourse import bass_utils

F32 = mybir.dt.float32
F32R = mybir.dt.float32r
F16 = mybir.dt.float16
AX = mybir.AxisListType
ALU = mybir.AluOpType
ACT_F = mybir.ActivationFunctionType

B, T, D, F, E = 2, 2048, 1024, 4096, 8
NTOK = B * T              # 4096
NCORES = 8
TOK_PER_CORE = NTOK // NCORES  # 512
DO = D // 128             # 8 d-blocks
FT = F // 128             # 32 f-tiles

_cache = {}


def _run(nc, in_maps, trace=False, **kw):
    return bass_utils.run_bass_kernel_spmd(
        nc, in_maps, core_ids=list(range(NCORES)), trace=trace, **kw
    )


# ----------------------------------------------------------------- router ---
def _build_router_inner(batched):
    """Per core: xT_sl [128, DO*512] fp32, Wr [128, DO*E], id8 [E,E]
    -> comb [128, TT*E] fp32 (comb[p, tt*E+e] for token tt*128+p)."""
    TT = TOK_PER_CORE // 128  # 4 token tiles
    nc = bacc.Bacc("TRN2", target_bir_lowering=False, debug=False)
    xT_d = nc.dram_tensor("xT_sl", [128, DO * TOK_PER_CORE], F32,
                          kind="ExternalInput").ap()
    wr_d = nc.dram_tensor("Wr", [128, DO * E], F32, kind="ExternalInput").ap()
    id_d = nc.dram_tensor("id8", [E, E], F32, kind="ExternalInput").ap()
    out_d = nc.dram_tensor("comb", [128, TT * E], F32, kind="ExternalOutput").ap()

    with tile.TileContext(nc) as tc:
        with (
            tc.tile_pool(name="pool", bufs=1) as pool,
            tc.tile_pool(name="work", bufs=2) as work,
            tc.tile_pool(name="psum", bufs=2, space="PSUM") as psum,
            tc.tile_pool(name="psw", bufs=1, space="PSUM") as psw,
        ):
            xT_sb = pool.tile([128, DO, TOK_PER_CORE], F32)
            wr_sb = pool.tile([128, DO, E], F32)
            id_sb = pool.tile([E, E], F32)
            warm_sb = pool.tile([128, 512], F16)
            comb_sb = pool.tile([128, TT, E], F32)

            # x load split across both HW DMA queues (5/3: the scalar queue
            # starts ~2.5us later behind the ACT table load); small on SW
            xv = xT_d.rearrange("p (o t) -> p o t", o=DO)
            nc.gpsimd.memset(warm_sb[:], 0.0)
            nc.sync.dma_start(xT_sb[:, 0:5, :], xv[:, 0:5, :])
            nc.scalar.dma_start(xT_sb[:, 5:8, :], xv[:, 5:8, :])
            nc.gpsimd.dma_start(wr_sb[:], wr_d.rearrange("p (o e) -> p o e", o=DO))
            nc.gpsimd.dma_start(id_sb[:], id_d[:])

            # PE clock ramp while DMAs land — one accumulation chain so the
            # matmuls run back-to-back (separate start/stop warms serialize
            # on the PSUM WAR and never establish continuous-busy)
            warm_ps = psw.tile([128, 512], F32)
            NWARM = 20
            for i in range(NWARM):
                nc.tensor.matmul(warm_ps[:], warm_sb[:, :128], warm_sb[:],
                                 start=(i == 0), stop=(i == NWARM - 1))

            # logits.T [E, tok] with Wr stationary, then PE-transpose each
            # 128-token tile back to [tok, E]
            lpT = psum.tile([E, TOK_PER_CORE], F32, tag="lpT")
            for do in range(DO):
                nc.tensor.matmul(
                    lpT[:], wr_sb[:, do, :], xT_sb[:, do, :],
                    start=(do == 0), stop=(do == DO - 1),
                )
            lsbT = pool.tile([E, TOK_PER_CORE], F32)
            nc.vector.tensor_copy(lsbT[:], lpT[:])

            l4 = pool.tile([128, TT, E], F32)
            for tt in range(TT):
                lp = psum.tile([128, E], F32, tag="lp")
                nc.tensor.transpose(
                    lp[:], lsbT[:, tt * 128:(tt + 1) * 128], id_sb[:]
                )
                nc.vector.tensor_copy(l4[:, tt, :], lp[:])

            if batched:
                # top-2 + renormalized softmax over all 4 token tiles at once
                shp = [128, TT, E]
                mx1 = work.tile([128, TT, 1], F32, tag="mx1")
                nc.vector.reduce_max(mx1[:], l4[:], axis=AX.X)
                mx1b = mx1[:].broadcast_to(shp)
                eq1 = work.tile(shp, F32, tag="eq1")
                nc.vector.tensor_tensor(eq1[:], l4[:], mx1b, op=ALU.is_equal)
                lm = work.tile(shp, F32, tag="lm")
                nc.vector.scalar_tensor_tensor(
                    lm[:], eq1[:], -1e30, l4[:], op0=ALU.mult, op1=ALU.add
                )
                mx2 = work.tile([128, TT, 1], F32, tag="mx2")
                nc.vector.reduce_max(mx2[:], lm[:], axis=AX.X)
                pd = work.tile(shp, F32, tag="pd")
                nc.vector.tensor_tensor(pd[:], l4[:], mx1b, op=ALU.subtract)
                p = work.tile(shp, F32, tag="p")
                nc.scalar.activation(p[:], pd[:], ACT_F.Exp)
                e2d = work.tile([128, TT, 1], F32, tag="e2d")
                nc.vector.tensor_tensor(e2d[:], mx2[:], mx1[:], op=ALU.subtract)
                e2 = work.tile([128, TT, 1], F32, tag="e2")
                nc.scalar.activation(e2[:], e2d[:], ACT_F.Exp)
                den = work.tile([128, TT, 1], F32, tag="den")
                nc.vector.tensor_scalar_add(den[:], e2[:], 1.0)
                rec = work.tile([128, TT, 1], F32, tag="rec")
                nc.vector.reciprocal(rec[:], den[:])
                ge = work.tile(shp, F32, tag="ge")
                nc.vector.tensor_tensor(ge[:], l4[:], mx2[:].broadcast_to(shp),
                                        op=ALU.is_ge)
                w = work.tile(shp, F32, tag="w")
                nc.vector.tensor_tensor(w[:], p[:], rec[:].broadcast_to(shp),
                                        op=ALU.mult)
                nc.vector.tensor_mul(comb_sb[:], w[:], ge[:])
            else:
                for tt in range(TT):
                    l = l4[:, tt, :]
                    mx1 = work.tile([128, 1], F32, tag="mx1")
                    nc.vector.reduce_max(mx1[:], l, axis=AX.X)
                    nmx1 = work.tile([128, 1], F32, tag="nmx1")
                    nc.vector.tensor_scalar_mul(nmx1[:], mx1[:], -1.0)
                    eq = work.tile([128, E], F32, tag="eq")
                    nc.vector.tensor_scalar(eq[:], l, mx1[:], None, op0=ALU.is_equal)
                    lm = work.tile([128, E], F32, tag="lm")
                    nc.vector.scalar_tensor_tensor(
                        lm[:], eq[:], -1e30, l, op0=ALU.mult, op1=ALU.add
                    )
                    mx2 = work.tile([128, 1], F32, tag="mx2")
                    nc.vector.reduce_max(mx2[:], lm[:], axis=AX.X)
                    p = work.tile([128, E], F32, tag="p")
                    nc.scalar.activation(p[:], l, ACT_F.Exp, bias=nmx1[:])
                    e2 = work.tile([128, 1], F32, tag="e2")
                    nc.scalar.activation(e2[:], mx2[:], ACT_F.Exp, bias=nmx1[:])
                    den = work.tile([128, 1], F32, tag="den")
                    nc.vector.tensor_scalar_add(den[:], e2[:], 1.0)
                    rec = work.tile([128, 1], F32, tag="rec")
                    nc.vector.reciprocal(rec[:], den[:])
                    ge = work.tile([128, E], F32, tag="ge")
                    nc.vector.tensor_scalar(ge[:], l, mx2[:], None, op0=ALU.is_ge)
                    w = work.tile([128, E], F32, tag="w")
                    nc.vector.tensor_scalar_mul(w[:], p[:], rec[:])
                    nc.vector.tensor_mul(comb_sb[:, tt, :], w[:], ge[:])

            nc.sync.dma_start(out_d.rearrange("p (t e) -> p t e", t=TT), comb_sb[:])
    nc.compile()
    return nc


def build_router():
    if "router" in _cache:
        return _cache["router"]
    try:
        nc = _build_router_inner(batched=True)
    except Exception:
        nc = _build_router_inner(batched=False)
    _cache["router"] = nc
    return nc


# -------------------------------------------------------------------- ffn ---
def build_ffn(cap, with_b1, with_b2):
    """Per core (expert e), all matmul operands fp16, accumulate fp32:
      xg [128, DO*cap], W1 [128, FT, DO, 128], W2 [128, FT, D] (+ b1/b2)
      -> y [128, TT, D] fp16 with y = cvec * (gelu(xg@W1 + b1) @ W2 + b2).
    Weights fully resident in SBUF; single pass over F per token chunk."""
    key = ("ffn", cap, with_b1, with_b2)
    if key in _cache:
        return _cache[key]
    assert cap % 32 == 0
    chunks = _chunk_split(cap)
    CH = max(cs for _, cs in chunks)
    TT = -(-cap // 128)

    nc = bacc.Bacc("TRN2", target_bir_lowering=False, debug=False)
    xg_d = nc.dram_tensor("xg", [128, DO * cap], F16, kind="ExternalInput").ap()
    w1_d = nc.dram_tensor("W1e", [128, FT, DO, 128], F16, kind="ExternalInput").ap()
    w2_d = nc.dram_tensor("W2e", [128, FT, D], F16, kind="ExternalInput").ap()
    b1_d = nc.dram_tensor("b1e", [128, FT], F32, kind="ExternalInput").ap()
    b2_d = nc.dram_tensor("b2e", [1, D], F16, kind="ExternalInput").ap()
    ones_d = nc.dram_tensor("ones", [1, 128], F16, kind="ExternalInput").ap()
    cv_d = nc.dram_tensor("cvec", [128, TT], F32, kind="ExternalInput").ap()
    y_d = nc.dram_tensor("y", [128, TT, D], F16, kind="ExternalOutput").ap()

    with tile.TileContext(nc) as tc:
        with (
            tc.tile_pool(name="res", bufs=1) as res,
            tc.tile_pool(name="outp", bufs=2) as outp,
            tc.tile_pool(name="ps1", bufs=4, space="PSUM") as ps1,
            tc.tile_pool(name="ps2", bufs=1, space="PSUM") as ps2,
        ):
            xg_sb = res.tile([128, DO * cap], F16)
            w1_sb = res.tile([128, FT, DO, 128], F16)
            w2_sb = res.tile([128, FT, D], F16)
            hT_sb = res.tile([128, FT, CH], F16)
            b1_sb = res.tile([128, FT], F32)
            b2_sb = res.tile([1, D], F16)
            ones_sb = res.tile([1, 128], F16)
            cv_sb = res.tile([128, TT], F32)
            warm_sb = res.tile([128, 512], F16)

            # --- input DMA schedule. Caution: dma_start BLOCKS the issuing
            # engine while its queue is full, and the scalar engine must be
            # free to run gelu from ~15us on — so scalar gets only 2 up-front
            # issues; most of W2 is issued from inside the chunk-0 loop,
            # paced between gelus. 4-ft W1 pieces give 8 KiB/partition
            # packets (~250 GB/s per HW queue), streaming just ahead of
            # chunk-0 stage 1.
            nc.gpsimd.memset(warm_sb[:], 0.0)
            cs0 = chunks[0][1]
            half = 4 * cs0
            nc.sync.dma_start(xg_sb[:, :half], xg_d[:, :half])
            nc.scalar.dma_start(xg_sb[:, half:2 * half], xg_d[:, half:2 * half])
            nc.sync.dma_start(w1_sb[:, 0:2], w1_d[:, 0:2])
            nc.scalar.dma_start(w1_sb[:, 2:4], w1_d[:, 2:4])
            nc.scalar.dma_start(w1_sb[:, 4:6], w1_d[:, 4:6])
            for k in [6, 8, 12, 16, 20, 24, 28]:
                nc.sync.dma_start(w1_sb[:, k:k + 2], w1_d[:, k:k + 2])
            nc.gpsimd.dma_start(cv_sb[:], cv_d[:])
            if with_b1:
                nc.gpsimd.dma_start(b1_sb[:], b1_d[:])
            if with_b2:
                nc.gpsimd.dma_start(b2_sb[:], b2_d[:])
                nc.gpsimd.dma_start(ones_sb[:], ones_d[:])
            for k in [10, 14, 18, 22, 26, 30]:
                nc.gpsimd.dma_start(w1_sb[:, k:k + 2], w1_d[:, k:k + 2])
            # tails: remaining x chunks and the last W2 pieces
            nc.sync.dma_start(w2_sb[:, 20:26], w2_d[:, 20:26])
            for c0, cs in chunks[1:]:
                nc.gpsimd.dma_start(
                    xg_sb[:, DO * c0:DO * (c0 + cs)], xg_d[:, DO * c0:DO * (c0 + cs)]
                )
            nc.gpsimd.dma_start(w2_sb[:, 26:FT], w2_d[:, 26:FT])

            # PE clock ramp while the first W1/xg DMAs land — single
            # accumulation chain for back-to-back execution (see router)
            warm_ps = ps1.tile([128, 512], F32, tag="hp", name="warm_ps",
                               bufs=4)
            NWARM = 28
            for i in range(NWARM):
                nc.tensor.matmul(warm_ps[:], warm_sb[:, :128], warm_sb[:],
                                 start=(i == 0), stop=(i == NWARM - 1))

            for ci, (c0, cs) in enumerate(chunks):
                ntt = -(-cs // 128)
                # stage 1: hT[f, tok] = gelu(W1.T @ x + b1), all of F
                for ft in range(FT):
                    hp = ps1.tile([128, 512], F32, tag="hp")
                    for do in range(DO):
                        nc.tensor.matmul(
                            hp[:, :cs],
                            w1_sb[:, ft, do, :],
                            xg_sb[:, DO * c0 + do * cs:DO * c0 + (do + 1) * cs],
                            start=(do == 0),
                            stop=(do == DO - 1),
                        )
                    if with_b1:
                        nc.scalar.activation(hT_sb[:, ft, :cs], hp[:, :cs],
                                             ACT_F.Gelu, bias=b1_sb[:, ft:ft + 1])
                    else:
                        nc.scalar.activation(hT_sb[:, ft, :cs], hp[:, :cs],
                                             ACT_F.Gelu)
                    # paced W2 issues on the (otherwise idle-ish) scalar
                    # engine during chunk-0 stage 1 — never >2 in flight
                    if ci == 0 and ft in (4, 8, 12, 16, 20):
                        nc.scalar.dma_start(w2_sb[:, ft - 4:ft],
                                            w2_d[:, ft - 4:ft])
                # stage 2: y[tok, d] = cvec * (hT.T @ W2 (+ b2)), fo-major over
                # pairs of token tiles so W2[fo] is consumed at half the rate
                # (relaxes the W2 streaming deadline on chunk 0)
                tiles = list(range(ntt))
                groups = [tiles[i:i + 2] for i in range(0, ntt, 2)]
                for group in groups:
                    yps, ms = [], []
                    for gi, tt in enumerate(group):
                        yps.append(ps2.tile([128, D], F32, tag=f"yp{gi}",
                                            name=f"yp{gi}"))
                        ms.append(min(128, cs - tt * 128))
                    for fo in range(FT):
                        for gi, tt in enumerate(group):
                            for n in range(2):
                                nc.tensor.matmul(
                                    yps[gi][:ms[gi], n * 512:(n + 1) * 512],
                                    hT_sb[:, fo, tt * 128:tt * 128 + ms[gi]],
                                    w2_sb[:, fo, n * 512:(n + 1) * 512],
                                    start=(fo == 0),
                                    stop=(fo == FT - 1 and not with_b2),
                                )
                    for gi, tt in enumerate(group):
                        m = ms[gi]
                        if with_b2:
                            for n in range(2):
                                nc.tensor.matmul(
                                    yps[gi][:m, n * 512:(n + 1) * 512],
                                    ones_sb[:, :m],
                                    b2_sb[:, n * 512:(n + 1) * 512],
                                    start=False,
                                    stop=True,
                                )
                        gt = c0 // 128 + tt
                        ot = outp.tile([128, D], F16, tag="ot")
                        nc.vector.tensor_scalar_mul(ot[:m, :], yps[gi][:m, :],
                                                    cv_sb[:m, gt:gt + 1])
                        nc.sync.dma_start(y_d[:m, gt, :], ot[:m, :])
    nc.compile()
    _cache[key] = nc
    return nc


# ---------------------------------------------------------------- combine ---
def build_combine():
    """Per core: packed a, b [128, (T/128)*D] fp16 -> o = a + b (fp16).

    Host packs A[t, d] -> Ah[p, tt*D + d] with t = tt*128 + p so every DMA is
    one contiguous segment per partition."""
    if "comb" in _cache:
        return _cache["comb"]
    W = (TOK_PER_CORE // 128) * D  # 4096
    PW = W // 2
    HP = PW // 2
    nc = bacc.Bacc("TRN2", target_bir_lowering=False, debug=False)
    a_d = nc.dram_tensor("a", [128, W], F16, kind="ExternalInput").ap()
    b_d = nc.dram_tensor("b", [128, W], F16, kind="ExternalInput").ap()
    o_d = nc.dram_tensor("o", [128, W], F16, kind="ExternalOutput").ap()
    with tile.TileContext(nc) as tc:
        with tc.tile_pool(name="pool", bufs=2) as pool:
            # 2 big read pieces per tensor (4 KiB/partition packets keep the
            # HW queues at full rate); adds per piece; outs on all 3 queues
            tiles = []
            for pc in range(2):
                sl = slice(pc * PW, (pc + 1) * PW)
                at = pool.tile([128, PW], F16, tag="a")
                bt = pool.tile([128, PW], F16, tag="b")
                nc.sync.dma_start(at[:], a_d[:, sl])
                nc.scalar.dma_start(bt[:], b_d[:, sl])
                nc.vector.tensor_add(at[:], at[:], bt[:])
                tiles.append(at)
            nc.gpsimd.dma_start(o_d[:, 0:PW], tiles[0][:])
            nc.sync.dma_start(o_d[:, PW:PW + HP], tiles[1][:, 0:HP])
            nc.scalar.dma_start(o_d[:, PW + HP:W], tiles[1][:, HP:PW])
    nc.compile()
    _cache["comb"] = nc
    return nc


# ----------------------------------------------------------------- driver ---
def _chunk_split(cap):
    """Split cap (multiple of 32) into chunks: all 128-aligned starts, sizes
    multiples of 128 except the last (multiple of 32), each <=512."""
    full = cap // 128
    rem = cap % 128
    k = -(-cap // 512)
    counts = [full // k + (1 if i < full % k else 0) for i in range(k)]
    chunks, c0 = [], 0
    for i, n in enumerate(counts):
        cs = n * 128 + (rem if i == k - 1 else 0)
        chunks.append((c0, cs))
        c0 += cs
    return chunks


def _moe_forward(x2d, Wr, W1, b1, W2, b2, trace=False):
    """x2d: [NTOK, D] fp32. Returns (out [NTOK, D] fp32, exec_ns_total|None)."""
    # --- launch 1: router ---
    rnc = build_router()
    wrh = np.ascontiguousarray(Wr.reshape(DO, 128, E).transpose(1, 0, 2).reshape(128, -1))
    id8 = np.eye(E, dtype=np.float32)
    in_maps = [
        {"xT_sl": np.ascontiguousarray(
            x2d[c * TOK_PER_CORE:(c + 1) * TOK_PER_CORE]
            .reshape(TOK_PER_CORE, DO, 128).transpose(2, 1, 0).reshape(128, -1)),
         "Wr": wrh, "id8": id8}
        for c in range(NCORES)
    ]
    rres = _run(rnc, in_maps, trace=trace)
    comb = np.concatenate(
        [rres.results[c]["comb"].reshape(128, TOK_PER_CORE // 128, E)
         .transpose(1, 0, 2).reshape(TOK_PER_CORE, E) for c in range(NCORES)],
        axis=0)
    exec_ns = rres.exec_time_ns or 0
    per_launch = [rres.exec_time_ns]

    # --- host dispatch (data movement only) ---
    x16 = x2d.astype(np.float16)
    top2 = np.argpartition(-comb, 1, axis=1)[:, :2]  # [NTOK, 2]
    sel_lists, cvals = [], []
    for e in range(E):
        sel = np.nonzero((top2 == e).any(axis=1))[0]
        sel_lists.append(sel)
        cvals.append(comb[sel, e])
    counts = np.array([len(s) for s in sel_lists])
    MAXCAP = 2048  # SBUF limit for resident x + weights
    nbatch = max(1, -(-int(counts.max()) // MAXCAP))
    cap = int(max(256, -(-(-(-counts.max() // nbatch)) // 32) * 32))

    with_b1 = bool(np.any(b1))
    with_b2 = bool(np.any(b2))
    fnc = build_ffn(cap, with_b1, with_b2)
    chunks = _chunk_split(cap)
    TTC = -(-cap // 128)
    ones_in = np.ones((1, 128), np.float16)
    w_packed = [
        {"W1e": np.ascontiguousarray(
            W1[e].astype(np.float16).reshape(DO, 128, FT, 128)
            .transpose(1, 2, 0, 3)),
         "b1e": np.ascontiguousarray(b1[e].reshape(FT, 128).T),
         "W2e": np.ascontiguousarray(
            W2[e].astype(np.float16).reshape(FT, 128, D).transpose(1, 0, 2)),
         "b2e": np.ascontiguousarray(b2[e].astype(np.float16)).reshape(1, D)}
        for e in range(E)
    ]
    ys = [np.zeros((0, D), np.float16) for _ in range(E)]
    for bi in range(nbatch):
        in_maps = []
        for e in range(E):
            sel_b = sel_lists[e][bi * cap:(bi + 1) * cap]
            cv_b = cvals[e][bi * cap:(bi + 1) * cap]
            n_e = len(sel_b)
            xsel = np.zeros((cap, D), np.float16)
            xsel[:n_e] = x16[sel_b]
            xg = np.concatenate(
                [xsel[c0:c0 + cs].reshape(cs, DO, 128).transpose(2, 1, 0)
                 .reshape(128, -1) for (c0, cs) in chunks], axis=1)
            cv = np.zeros(TTC * 128, np.float32)
            cv[:n_e] = cv_b
            cv = np.ascontiguousarray(cv.reshape(TTC, 128).T)
            in_maps.append({"xg": np.ascontiguousarray(xg), "ones": ones_in,
                            "cvec": cv, **w_packed[e]})
        fres = _run(fnc, in_maps, trace=trace)
        ys = [np.concatenate(
            [ys[e],
             fres.results[e]["y"].reshape(128, TTC, D).transpose(1, 0, 2)
             .reshape(TTC * 128, D)[:cap]]) for e in range(E)]
        exec_ns += fres.exec_time_ns or 0
        per_launch.append(fres.exec_time_ns)

    # --- host: build per-token (A, B) contribution rows (gather only) ---
    slot = np.zeros((NTOK, E), np.int64)
    for e in range(E):
        slot[sel_lists[e], e] = np.arange(counts[e])
    e1, e2v = top2[:, 0], top2[:, 1]
    A = np.empty((NTOK, D), np.float16)
    Bm = np.empty((NTOK, D), np.float16)
    for e in range(E):
        m1 = e1 == e
        A[m1] = ys[e][slot[m1, e]]
        m2 = e2v == e
        Bm[m2] = ys[e][slot[m2, e]]

    # --- launch 3: combine ---
    cnc = build_combine()

    def pack(m, c):
        sl = m[c * TOK_PER_CORE:(c + 1) * TOK_PER_CORE]
        return np.ascontiguousarray(
            sl.reshape(TOK_PER_CORE // 128, 128, D).transpose(1, 0, 2)
            .reshape(128, -1))

    in_maps = [{"a": pack(A, c), "b": pack(Bm, c)} for c in range(NCORES)]
    cres = _run(cnc, in_maps, trace=trace)
    out = np.concatenate(
        [cres.results[c]["o"].reshape(128, TOK_PER_CORE // 128, D)
         .transpose(1, 0, 2).reshape(TOK_PER_CORE, D) for c in range(NCORES)],
        axis=0).astype(np.float32)
    exec_ns += cres.exec_time_ns or 0
    per_launch.append(cres.exec_time_ns)
    if trace:
        print(f"per-launch exec ns (router, ffn, combine): {per_launch}")
        _moe_forward.last = (rres, fres, cres)
    return out, (exec_ns if trace else None)


def kernel(x, Wr, W1, b1, W2, b2):
    x = np.asarray(x, np.float32)
    out, _ = _moe_forward(
        x.reshape(NTOK, D),
        np.asarray(Wr, np.float32),
        np.asarray(W1, np.float32),
        np.asarray(b1, np.float32),
        np.asarray(W2, np.float32),
        np.asarray(b2, np.float32),
        trace=False,
    )
    return out.reshape(B, T, D)
